# revision 2
# baseline (speedup 1.0000x reference)
"""Trainium2 Bass kernel for nn_CmxuLayer: y = U.T @ X, U = 6x6 complex unitary
built from 36 phases, X = [6, 2097152] complex64 given as separate re/im f32 planes.

Strategy (pure data parallel over 8 NeuronCores):
  - Host builds the 6x6 unitary U from the phases (negligible), and packs it into a
    real [120, 120] stationary matrix W implementing the complex matmul on 10
    batch-groups at once (120 = 12 re/im channel components x 10 groups).
  - The kernel is HBM-bandwidth bound (~358 GB/s/core, reads+writes combined).
    All device IO is fp16: the harness tolerance is rel_err < 2e-2 and fp16
    quantization of randn data costs ~2e-4 through a unitary, so halving the
    bytes is free accuracy-wise and halves the roofline.
  - Host packs re+im into ONE [12, B_PAD] fp16 tensor per direction (rows 0-5 re,
    6-11 im), so each super-tile needs a single 120-partition input DMA and a
    single output DMA instead of two 60-partition ones.
  - Each core gets a contiguous batch shard of 262144 columns, zero-padded to
    262150 and reshaped to 10 groups x 26215. The moving operand is [120, N]
    fp16 in SBUF.
  - One fp16 PE matmul per 512 columns -> PSUM [120, 512] f32; DVE/ACT copy
    casts to fp16 in SBUF; SWDGE DMA out. Host re-assembles complex64 on gather.
  - Each data stream gets its own issuing engine: input DMAs on the SP HWDGE
    ring, output DMAs on the GpSimd SWDGE ring, PSUM copies split across
    DVE/ACT - so no stream's stall can head-of-line-block another's issue.
"""

import numpy as np

N_CH = 6
BATCH = 2097152
N_CORES = 8
B_CORE = BATCH // N_CORES      # 262144 true columns per core
G = 10                         # batch groups per core (packed in partition dim)
NG = 26215                     # padded columns per group (= ceil(262144/10))
B_PAD = G * NG                 # 262150 padded columns per core (6 pad cols)
K = 12 * G                     # 120 partitions
TILE_N = 512                   # matmul free dim (one PSUM bank @ fp32)
ST = 2048                      # per-group columns per super-tile (DMA granularity)

_CACHE = {}


def _build_unitary(mzi_phases, output_phases):
    """Mirror reference.build_unitary in numpy (f32/c64 arithmetic)."""
    n = N_CH
    U = np.eye(n, dtype=np.complex64)
    idx = 0
    mz = np.asarray(mzi_phases, np.float32)
    op = np.asarray(output_phases, np.float32)
    j1 = np.complex64(1j)
    for i in range(n):
        for j in range(i + 1, n):
            theta = mz[idx]
            phi = mz[idx + 1]
            idx += 2
            c = np.complex64(np.cos(theta))
            s = np.complex64(np.sin(theta))
            eip = np.exp(j1 * phi).astype(np.complex64)
            row_i = eip * c * U[i] + s * U[j]
            row_j = -eip * s * U[i] + c * U[j]
            U = U.copy()
            U[i] = row_i
            U[j] = row_j
    U = np.exp(j1 * op)[:, None].astype(np.complex64) * U
    return U


def _build_weights(U):
    """Pack U into the [K, K] f32 stationary lhsT.

    matmul computes out[m, n] = sum_k lhsT[k, m] * rhs[k, n].
    rhs partition k = ci*G + g holds xr[ci] of group g (ci in 0..5),
                 k = (6+ci)*G + g holds xi[ci] of group g.
    out partition m = c*G + g is y_re[c] of group g,
                  m = (6+c)*G + g is y_im[c] of group g.
    y = U.T x  =>  y[c] = sum_ci U[ci, c] x[ci].
    """
    Ur = np.ascontiguousarray(U.real.astype(np.float32))
    Ui = np.ascontiguousarray(U.imag.astype(np.float32))
    W = np.zeros((K, K), np.float32)
    for g in range(G):
        for ci in range(N_CH):
            for c in range(N_CH):
                W[ci * G + g, c * G + g] = Ur[ci, c]
                W[(6 + ci) * G + g, c * G + g] = -Ui[ci, c]
                W[ci * G + g, (6 + c) * G + g] = Ui[ci, c]
                W[(6 + ci) * G + g, (6 + c) * G + g] = Ur[ci, c]
    return W


def _get_compiled(reps=1, variant="v4", st=None):
    if st is None:
        st = ST
    key = ("nc", reps, variant, st)
    if key in _CACHE:
        return _CACHE[key]

    import concourse.bass as bass
    import concourse.mybir as mybir
    from concourse import bacc
    from concourse.bass import ds, ts
    from concourse.tile import TileContext

    f32 = mybir.dt.float32
    f16 = mybir.dt.float16
    nc = bacc.Bacc(
        trn_type="TRN2",
        target_bir_lowering=False,
        debug=False,
        num_devices=N_CORES,
    )
    xb = nc.dram_tensor("xb", [12, B_PAD], f16, kind="ExternalInput").ap()
    w = nc.dram_tensor("w", [K, K], f16, kind="ExternalInput").ap()
    yb = nc.dram_tensor("yb", [12, B_PAD], f16, kind="ExternalOutput").ap()

    xb_r = xb.rearrange("c (g n) -> c g n", g=G)
    yb_r = yb.rearrange("c (g n) -> c g n", g=G)

    n_bufs = 6
    with TileContext(nc) as tc:
        with (
            tc.tile_pool(name="wpool", bufs=1) as wp,
            tc.tile_pool(name="mv", bufs=n_bufs) as mvp,
            tc.tile_pool(name="ot", bufs=n_bufs) as op,
            tc.tile_pool(name="ps", bufs=8, space="PSUM") as pp,
        ):
            wt = wp.tile([K, K], f16)
            nc.sync.dma_start(out=wt[:], in_=w[:])

            # full st-col super-tiles + remainder = NG cols/group
            st_list = []
            off = 0
            while off < NG:
                stn = min(st, NG - off)
                st_list.append((off, stn))
                off += stn

            def body():
                for off, stn in st_list:
                    mv = mvp.tile([K, stn], f16, tag="mv")
                    nc.sync.dma_start(out=mv[:, :], in_=xb_r[:, :, ds(off, stn)])
                    if variant == "dma":
                        # stream straight back out, skipping compute
                        nc.scalar.dma_start(
                            out=yb_r[:, :, ds(off, stn)], in_=mv[:, :]
                        )
                        continue
                    ot = op.tile([K, stn], f16, tag="ot")
                    for j in range((stn + TILE_N - 1) // TILE_N):
                        nj = min(TILE_N, stn - j * TILE_N)
                        ps = pp.tile([K, TILE_N], f32, tag="ps")
                        nc.tensor.matmul(
                            out=ps[:, 0:nj],
                            lhsT=wt[:],
                            rhs=mv[:, ds(j * TILE_N, nj)],
                            start=True,
                            stop=True,
                        )
                        if j % 2 == 0:
                            nc.vector.tensor_copy(
                                out=ot[:, ds(j * TILE_N, nj)], in_=ps[:, 0:nj]
                            )
                        else:
                            nc.scalar.copy(
                                out=ot[:, ds(j * TILE_N, nj)], in_=ps[:, 0:nj]
                            )
                    # Output DMAs on the SWDGE (GpSimd) ring so they neither
                    # head-of-line-block the SP ring's next input DMA nor ACT's
                    # next-tile copies.
                    nc.gpsimd.dma_start(out=yb_r[:, :, ds(off, stn)], in_=ot[:])

            if reps == 1:
                body()
            else:
                with tc.For_i(0, reps, 1):
                    body()

    nc.compile()
    _CACHE[key] = nc
    return nc


def _make_in_maps(field_re, field_im, W):
    """Per-core input dicts: fp16 packed [12, B_PAD] shard + fp16 weights."""
    W16 = np.ascontiguousarray(W.astype(np.float16))
    maps = []
    for i in range(N_CORES):
        sl = slice(i * B_CORE, (i + 1) * B_CORE)
        xbv = np.zeros((12, B_PAD), np.float16)
        xbv[0:N_CH, :B_CORE] = field_re[:, sl]
        xbv[N_CH:, :B_CORE] = field_im[:, sl]
        maps.append({"xb": xbv, "w": W16})
    return maps


def kernel(field_re, field_im, mzi_phases, output_phases):
    from concourse import bass_utils

    field_re = np.asarray(field_re)
    field_im = np.asarray(field_im)
    U = _build_unitary(mzi_phases, output_phases)
    W = _build_weights(U)

    nc = _get_compiled(variant="v4")
    in_maps = _make_in_maps(field_re, field_im, W)
    res = bass_utils.run_bass_kernel_spmd(nc, in_maps, core_ids=list(range(N_CORES)))

    out = np.empty((N_CH, BATCH), np.complex64)
    for i in range(N_CORES):
        sl = slice(i * B_CORE, (i + 1) * B_CORE)
        ybv = res.results[i]["yb"]
        out.real[:, sl] = ybv[0:N_CH, :B_CORE]
        out.imag[:, sl] = ybv[N_CH:, :B_CORE]
    return out


# revision 18
# speedup vs baseline: 1.2744x; 1.2744x over previous
"""Trainium2 Bass kernel for nn_CmxuLayer: y = U.T @ X, U = 6x6 complex unitary
built from 36 phases, X = [6, 2097152] complex64 given as separate re/im f32 planes.

Strategy (pure data parallel over 8 NeuronCores):
  - Host builds the 6x6 unitary U from the phases (negligible), and packs it into a
    real [120, 120] stationary matrix W implementing the complex matmul on 10
    batch-groups at once (120 = 12 re/im channel components x 10 groups).
  - The kernel is HBM-bandwidth bound (~358 GB/s/core, reads+writes combined).
    All device IO is fp16: the harness tolerance is rel_err < 2e-2 and fp16
    quantization of randn data costs ~2e-4 through a unitary, so halving the
    bytes is free accuracy-wise and halves the roofline.
  - Host packs re+im into ONE [12, B_PAD] fp16 tensor per direction (rows 0-5 re,
    6-11 im), so each super-tile needs a single 120-partition input DMA and a
    single output DMA instead of two 60-partition ones.
  - Each core gets a contiguous batch shard of 262144 columns, zero-padded to
    262150 and reshaped to 10 groups x 26215. The moving operand is [120, N]
    fp16 in SBUF, streamed in 8192-column super-tiles (4 in + 4 out DMAs/iter).
  - One fp16 PE matmul per 512 columns -> PSUM [120, 512] f32; DVE copies cast
    to fp16 in SBUF; SWDGE DMA out. Host re-assembles complex64 on gather.
  - DMA issue is spread across all three issue paths (v5): input DMAs
    alternate between the SP and ACT HWDGE rings, output DMAs ride the GpSimd
    SWDGE ring. Three concurrently-active queues keep more packets outstanding
    in the 16 SDMA engines (which round-robin across queues), measurably
    raising effective HBM bandwidth vs. any single- or two-queue scheme.
  - The For_i reps loop used for steady-state timing unrolls UB=8 bodies per
    iteration to amortize the all-engine barrier For_i emits per iteration.
"""

import numpy as np

N_CH = 6
BATCH = 2097152
N_CORES = 8
B_CORE = BATCH // N_CORES      # 262144 true columns per core
G = 10                         # batch groups per core (packed in partition dim)
NG = 26215                     # padded columns per group (= ceil(262144/10))
B_PAD = G * NG                 # 262150 padded columns per core (6 pad cols)
K = 12 * G                     # 120 partitions
TILE_N = 512                   # matmul free dim (one PSUM bank @ fp32)
ST = 8192                      # per-group columns per super-tile (DMA granularity)
UB = 8                         # bodies per For_i iteration (amortizes the
                               # all-engine barrier each For_i iteration emits)
VARIANT = "v5"                 # production engine-assignment variant

_CACHE = {}


def _build_unitary(mzi_phases, output_phases):
    """Mirror reference.build_unitary in numpy (f32/c64 arithmetic)."""
    n = N_CH
    U = np.eye(n, dtype=np.complex64)
    idx = 0
    mz = np.asarray(mzi_phases, np.float32)
    op = np.asarray(output_phases, np.float32)
    j1 = np.complex64(1j)
    for i in range(n):
        for j in range(i + 1, n):
            theta = mz[idx]
            phi = mz[idx + 1]
            idx += 2
            c = np.complex64(np.cos(theta))
            s = np.complex64(np.sin(theta))
            eip = np.exp(j1 * phi).astype(np.complex64)
            row_i = eip * c * U[i] + s * U[j]
            row_j = -eip * s * U[i] + c * U[j]
            U = U.copy()
            U[i] = row_i
            U[j] = row_j
    U = np.exp(j1 * op)[:, None].astype(np.complex64) * U
    return U


def _build_weights(U):
    """Pack U into the [K, K] f32 stationary lhsT.

    matmul computes out[m, n] = sum_k lhsT[k, m] * rhs[k, n].
    rhs partition k = ci*G + g holds xr[ci] of group g (ci in 0..5),
                 k = (6+ci)*G + g holds xi[ci] of group g.
    out partition m = c*G + g is y_re[c] of group g,
                  m = (6+c)*G + g is y_im[c] of group g.
    y = U.T x  =>  y[c] = sum_ci U[ci, c] x[ci].
    """
    Ur = np.ascontiguousarray(U.real.astype(np.float32))
    Ui = np.ascontiguousarray(U.imag.astype(np.float32))
    W = np.zeros((K, K), np.float32)
    for g in range(G):
        for ci in range(N_CH):
            for c in range(N_CH):
                W[ci * G + g, c * G + g] = Ur[ci, c]
                W[(6 + ci) * G + g, c * G + g] = -Ui[ci, c]
                W[ci * G + g, (6 + c) * G + g] = Ui[ci, c]
                W[(6 + ci) * G + g, (6 + c) * G + g] = Ur[ci, c]
    return W


def _get_compiled(reps=1, variant="v4", st=None, unroll=False, ub=None):
    if st is None:
        st = ST
    if ub is None:
        ub = UB if (reps > 1 and not unroll and reps % UB == 0) else 1
    key = ("nc", reps, variant, st, unroll, ub)
    if key in _CACHE:
        return _CACHE[key]

    import concourse.bass as bass
    import concourse.mybir as mybir
    from concourse import bacc
    from concourse.bass import ds, ts
    from concourse.tile import TileContext

    f32 = mybir.dt.float32
    f16 = mybir.dt.float16
    nc = bacc.Bacc(
        trn_type="TRN2",
        target_bir_lowering=False,
        debug=False,
        num_devices=N_CORES,
    )
    xb = nc.dram_tensor("xb", [12, B_PAD], f16, kind="ExternalInput").ap()
    w = nc.dram_tensor("w", [K, K], f16, kind="ExternalInput").ap()
    yb = nc.dram_tensor("yb", [12, B_PAD], f16, kind="ExternalOutput").ap()

    xb_r = xb.rearrange("c (g n) -> c g n", g=G)
    yb_r = yb.rearrange("c (g n) -> c g n", g=G)

    # fit mv (+ot for compute variants) double-buffering into ~160KB/partition
    per_buf = st * 2 * (1 if variant == "dma" else 2)
    n_bufs = min(6, max(2, (160 * 1024) // per_buf))
    with TileContext(nc) as tc:
        with (
            tc.tile_pool(name="wpool", bufs=1) as wp,
            tc.tile_pool(name="mv", bufs=n_bufs) as mvp,
            tc.tile_pool(name="ot", bufs=n_bufs) as op,
            tc.tile_pool(name="ps", bufs=8, space="PSUM") as pp,
        ):
            wt = wp.tile([K, K], f16)
            nc.sync.dma_start(out=wt[:], in_=w[:])

            # full st-col super-tiles + remainder = NG cols/group
            st_list = []
            off = 0
            while off < NG:
                stn = min(st, NG - off)
                st_list.append((off, stn))
                off += stn

            V5 = ("v5", "v5b", "v5c")

            def body():
                for ti, (off, stn) in enumerate(st_list):
                    mv = mvp.tile([K, stn], f16, tag="mv")
                    # v5*: alternate input issue between the two HWDGE rings
                    idma = nc.scalar if (variant in V5 and ti % 2) else nc.sync
                    idma.dma_start(out=mv[:, :], in_=xb_r[:, :, ds(off, stn)])
                    if variant == "dma":
                        # stream straight back out, skipping compute
                        nc.scalar.dma_start(
                            out=yb_r[:, :, ds(off, stn)], in_=mv[:, :]
                        )
                        continue
                    ot = op.tile([K, stn], f16, tag="ot")
                    for j in range((stn + TILE_N - 1) // TILE_N):
                        nj = min(TILE_N, stn - j * TILE_N)
                        ps = pp.tile([K, TILE_N], f32, tag="ps")
                        nc.tensor.matmul(
                            out=ps[:, 0:nj],
                            lhsT=wt[:],
                            rhs=mv[:, ds(j * TILE_N, nj)],
                            start=True,
                            stop=True,
                        )
                        # v5/v5c/v6: all PSUM->SBUF copies on DVE (no ACT
                        # copies), freeing the ACT sequencer to issue DMAs;
                        # v4/v5b/v7 split copies across DVE and ACT.
                        if variant in ("v5", "v5c", "v6") or j % 2 == 0:
                            nc.vector.tensor_copy(
                                out=ot[:, ds(j * TILE_N, nj)], in_=ps[:, 0:nj]
                            )
                        else:
                            nc.scalar.copy(
                                out=ot[:, ds(j * TILE_N, nj)], in_=ps[:, 0:nj]
                            )
                    # v4/v5/v5b: outputs on the SWDGE (GpSimd) ring.
                    # v6/v7: outputs on the ACT HWDGE ring.
                    # v5c: alternate outputs between SWDGE and ACT HWDGE.
                    if variant in ("v6", "v7"):
                        odma = nc.scalar
                    elif variant == "v5c" and ti % 2:
                        odma = nc.scalar
                    else:
                        odma = nc.gpsimd
                    odma.dma_start(out=yb_r[:, :, ds(off, stn)], in_=ot[:])

            if reps == 1:
                body()
            elif unroll:
                for _ in range(reps):
                    body()
            else:
                # For_i emits an all-engine barrier per iteration, which
                # drains the DMA pipeline (~10us exposed). Unroll ub bodies
                # per iteration to amortize it.
                assert reps % ub == 0, (reps, ub)
                with tc.For_i(0, reps // ub, 1):
                    for _ in range(ub):
                        body()

    nc.compile()
    _CACHE[key] = nc
    return nc


def _make_in_maps(field_re, field_im, W):
    """Per-core input dicts: fp16 packed [12, B_PAD] shard + fp16 weights."""
    W16 = np.ascontiguousarray(W.astype(np.float16))
    maps = []
    for i in range(N_CORES):
        sl = slice(i * B_CORE, (i + 1) * B_CORE)
        xbv = np.zeros((12, B_PAD), np.float16)
        xbv[0:N_CH, :B_CORE] = field_re[:, sl]
        xbv[N_CH:, :B_CORE] = field_im[:, sl]
        maps.append({"xb": xbv, "w": W16})
    return maps


def kernel(field_re, field_im, mzi_phases, output_phases):
    from concourse import bass_utils

    field_re = np.asarray(field_re)
    field_im = np.asarray(field_im)
    U = _build_unitary(mzi_phases, output_phases)
    W = _build_weights(U)

    nc = _get_compiled(variant=VARIANT)
    in_maps = _make_in_maps(field_re, field_im, W)
    res = bass_utils.run_bass_kernel_spmd(nc, in_maps, core_ids=list(range(N_CORES)))

    out = np.empty((N_CH, BATCH), np.complex64)
    for i in range(N_CORES):
        sl = slice(i * B_CORE, (i + 1) * B_CORE)
        ybv = res.results[i]["yb"]
        out.real[:, sl] = ybv[0:N_CH, :B_CORE]
        out.imag[:, sl] = ybv[N_CH:, :B_CORE]
    return out
